# revision 4
# baseline (speedup 1.0000x reference)
"""Mixtral sparse-MoE block with per-expert LoRA adapters on 8 Trainium2 cores.

Problem shapes: B=2, S=1024, H=2048, F=7168, E=8, R=32, top-K=2.
T = B*S = 2048 tokens.

Sharding: tensor-parallel over the FFN dim F. Core c owns rows
[c*896:(c+1)*896] of W1/W3 (and the matching B1/B3 LoRA rows) and the same
columns of W2/A2. Everything after the silu is linear in
x2s = silu(x1)*x3*rw, so each core emits an exact partial [H, T] output over
its F-shard and the host sums the 8 partials.

Work split (device vs host):
- Host: gating (softmax + top-2; 34M MACs), the tiny per-expert LoRA
  down-projections a1/a3 = x @ A{1,3}T masked per slot (m1/m3 uploads), and
  the final LoRA up-projection lora2 = B2 @ sum_cores(m2) (one small GEMM) -
  all the terminal-linear or pre-device pieces.
- Device (per core): base1/base3 = x @ W{1,3}[shard].T, the per-slot LoRA
  up-projections lora1/3 = B{1,3}[shard] @ m{1,3} accumulated onto the base
  in PSUM, silu/mul/scale chain, a2 = A2[shard] @ x2s with per-slot masking
  (m2, returned to host), and the down-projection on the slot-summed
  activations: down = W2[shard] @ (x2s_0 + x2s_1) - the two top-k slots share
  one down-proj because down is linear in x2s.

Layout is feature-major ([feature, token]) end to end so every matmul has its
contraction on the partition axis with zero on-device transposes. All matmul
operands are bf16 (fp32 matmul is 4x slower on TRN2); PSUM accumulates fp32.

Perf notes (measured on trn2 via an on-device For_i repeat loop):
- Each InstMatmult costs ~295ns at N=512 on this toolchain (LDWEIGHTS is
  emitted per-matmul by tile_legalize and does not overlap the matmul), so
  minimizing matmul count dominates: 1680 MMs -> ~475us/core.
- Every DMA instruction costs ~625ns on the shared HWDGE descriptor device,
  so weights are SBUF-resident and streamed tiles are [128, >=512].
"""

import sys
from contextlib import ExitStack

import numpy as np

try:
    import concourse.bass as bass  # noqa: F401
except ImportError:
    sys.path.insert(0, "/opt/trn_rl_repo")

import ml_dtypes

import concourse.bass as bass
import concourse.mybir as mybir
import concourse.tile as tile
from concourse import bacc
from concourse.bass_utils import run_bass_kernel_spmd

BF16 = mybir.dt.bfloat16
F32 = mybir.dt.float32
NPBF16 = ml_dtypes.bfloat16

B, S, H, F, E, R, K = 2, 1024, 2048, 7168, 8, 32, 2
T = B * S                      # 2048 tokens
ER = E * R                     # 256
NCORES = 8
FS = F // NCORES               # 896 per-core F shard
NH = H // 128                  # 16 h-chunks
NF = FS // 128                 # 7 f-chunks (per core)
NER = ER // 128                # 2 er-chunks
TBLK = 512
NT = T // TBLK                 # 4 token blocks


def build_nc(repeat=None):
    """Build the per-core Bass module.

    repeat=None emits the plain single-shot kernel (what the grader runs).
    repeat=n wraps the ENTIRE body — resident weight loads included — in an
    on-device For_i hardware loop for slope-based timing in test.py.
    """
    nc = bacc.Bacc(None)

    xT = nc.declare_dram_parameter("xT", [NH, 128, T], BF16, isOutput=False)
    w1t = nc.declare_dram_parameter("w1t", [NH, 128, FS], BF16, isOutput=False)
    w3t = nc.declare_dram_parameter("w3t", [NH, 128, FS], BF16, isOutput=False)
    w2t = nc.declare_dram_parameter("w2t", [NF, 128, H], BF16, isOutput=False)
    m1t = nc.declare_dram_parameter("m1t", [K, NER, 128, T], BF16, isOutput=False)
    m3t = nc.declare_dram_parameter("m3t", [K, NER, 128, T], BF16, isOutput=False)
    b1t = nc.declare_dram_parameter("b1t", [NER, 128, FS], BF16, isOutput=False)
    b3t = nc.declare_dram_parameter("b3t", [NER, 128, FS], BF16, isOutput=False)
    a2t = nc.declare_dram_parameter("a2t", [NF, 128, ER], BF16, isOutput=False)
    maskt = nc.declare_dram_parameter("maskt", [K, NER, 128, T], BF16, isOutput=False)
    rwr = nc.declare_dram_parameter("rwr", [K, 1, T], F32, isOutput=False)
    outT = nc.declare_dram_parameter("outT", [NH, 128, T], F32, isOutput=True)
    m2o = nc.declare_dram_parameter("m2o", [K, NER, 128, T], BF16, isOutput=True)

    with tile.TileContext(nc) as tc, ExitStack() as ctx:
        resw = ctx.enter_context(tc.tile_pool(name="resw", bufs=1))
        xsp = ctx.enter_context(tc.tile_pool(name="xsp", bufs=2))
        actp = ctx.enter_context(tc.tile_pool(name="actp", bufs=1))
        mp_ = ctx.enter_context(tc.tile_pool(name="mp", bufs=1))
        trans = ctx.enter_context(tc.tile_pool(name="trans", bufs=4))
        outp = ctx.enter_context(tc.tile_pool(name="outp", bufs=4))
        psp = ctx.enter_context(tc.tile_pool(name="psp", bufs=2, space="PSUM"))

        loop_cm = tc.For_i(0, repeat, 1) if repeat is not None else None
        if loop_cm is not None:
            loop_cm.__enter__()

        # ---- per-block input streamers ----
        def load_block_inputs(tb, xs=None):
            tsl = slice(tb * TBLK, (tb + 1) * TBLK)
            if xs is None:
                xs = []
                for h in range(NH):
                    xt_ = xsp.tile([128, TBLK], BF16, name=f"x{h}", tag=f"x{h}")
                    nc.sync.dma_start(out=xt_, in_=xT[h][:, tsl])
                    xs.append(xt_)
            msk = [[None] * NER for _ in range(K)]
            m1 = [[None] * NER for _ in range(K)]
            m3 = [[None] * NER for _ in range(K)]
            for k in range(K):
                for er in range(NER):
                    m_ = mp_.tile([128, TBLK], BF16, name=f"mk{k}_{er}",
                                  tag=f"mk{k}_{er}")
                    nc.sync.dma_start(out=m_, in_=maskt[k][er][:, tsl])
                    msk[k][er] = m_
                    m1_ = mp_.tile([128, TBLK], BF16, name=f"m1_{k}{er}",
                                   tag=f"m1_{k}{er}")
                    nc.sync.dma_start(out=m1_, in_=m1t[k][er][:, tsl])
                    m1[k][er] = m1_
                    m3_ = mp_.tile([128, TBLK], BF16, name=f"m3_{k}{er}",
                                   tag=f"m3_{k}{er}")
                    nc.sync.dma_start(out=m3_, in_=m3t[k][er][:, tsl])
                    m3[k][er] = m3_
            rws = []
            for k in range(K):
                r_ = mp_.tile([128, TBLK], F32, name=f"rw{k}", tag=f"rw{k}")
                nc.sync.dma_start(out=r_, in_=rwr[k][:, tsl].to_broadcast([128, TBLK]))
                rws.append(r_)
            return xs, msk, rws, m1, m3

        # ---- resident weights, emitted in first-use order so the HWDGE
        # queue feeds phase A of block 0 as early as possible ----
        def resident(src, n, shape, nm):
            ts = []
            for i in range(n):
                t_ = resw.tile(shape, BF16, name=f"{nm}{i}", tag=f"{nm}{i}")
                nc.sync.dma_start(out=t_, in_=src[i])
                ts.append(t_)
            return ts

        xs0 = []
        w1s, w3s = [], []
        for h in range(NH):
            xt_ = xsp.tile([128, TBLK], BF16, name=f"x{h}", tag=f"x{h}")
            nc.sync.dma_start(out=xt_, in_=xT[h][:, 0:TBLK])
            xs0.append(xt_)
            t1 = resw.tile([128, FS], BF16, name=f"w1s{h}", tag=f"w1s{h}")
            nc.sync.dma_start(out=t1, in_=w1t[h])
            w1s.append(t1)
            t3 = resw.tile([128, FS], BF16, name=f"w3s{h}", tag=f"w3s{h}")
            nc.sync.dma_start(out=t3, in_=w3t[h])
            w3s.append(t3)
        pre0 = load_block_inputs(0, xs0)
        b1s = resident(b1t, NER, [128, FS], "b1s")
        b3s = resident(b3t, NER, [128, FS], "b3s")
        w2s = resident(w2t, NF, [128, H], "w2s")
        a2s = resident(a2t, NF, [128, ER], "a2s")

        for tb in range(NT):
            tsl = slice(tb * TBLK, (tb + 1) * TBLK)
            xs, msk, rws, m1, m3 = pre0 if tb == 0 else load_block_inputs(tb)

            # ---- phase A: shared base1/base3 and LoRA down-projections ----
            base1 = [None] * NF
            base3 = [None] * NF
            for f in range(NF):
                fsl = slice(f * 128, (f + 1) * 128)
                ps1 = psp.tile([128, TBLK], F32, name="ps1", tag="pA")
                ps3 = psp.tile([128, TBLK], F32, name="ps3", tag="pB")
                for h in range(NH):
                    nc.tensor.matmul(ps1, w1s[h][:, fsl], xs[h], start=(h == 0), stop=(h == NH - 1))
                    nc.tensor.matmul(ps3, w3s[h][:, fsl], xs[h], start=(h == 0), stop=(h == NH - 1))
                b1_ = actp.tile([128, TBLK], BF16, name=f"b1_{f}", tag=f"b1_{f}")
                nc.scalar.copy(b1_, ps1)
                base1[f] = b1_
                b3_ = actp.tile([128, TBLK], BF16, name=f"b3_{f}", tag=f"b3_{f}")
                nc.scalar.copy(b3_, ps3)
                base3[f] = b3_

            # ---- phase B: per-slot LoRA up-proj, silu, x2s; then a2 ----
            x2s = [[None] * NF for _ in range(K)]
            xsum = [None] * NF
            m2 = [[None] * NER for _ in range(K)]
            for k in range(K):
                for f in range(NF):
                    fsl = slice(f * 128, (f + 1) * 128)
                    psA = psp.tile([128, TBLK], F32, name="psA", tag="pA")
                    nc.tensor.matmul(psA, b1s[0][:, fsl], m1[k][0], start=True, stop=False)
                    nc.tensor.matmul(psA, b1s[1][:, fsl], m1[k][1], start=False, stop=True)
                    psB = psp.tile([128, TBLK], F32, name="psB", tag="pB")
                    nc.tensor.matmul(psB, b3s[0][:, fsl], m3[k][0], start=True, stop=False)
                    nc.tensor.matmul(psB, b3s[1][:, fsl], m3[k][1], start=False, stop=True)
                    t1_ = trans.tile([128, TBLK], BF16, name="t1", tag="t1")
                    nc.vector.tensor_add(t1_, psA, base1[f])
                    sl_ = trans.tile([128, TBLK], BF16, name="sl", tag="sl")
                    nc.scalar.activation(sl_, t1_, mybir.ActivationFunctionType.Silu)
                    t3_ = trans.tile([128, TBLK], BF16, name="t3", tag="t3")
                    nc.vector.tensor_add(t3_, psB, base3[f])
                    x3s_ = trans.tile([128, TBLK], BF16, name="x3s", tag="x3s")
                    nc.vector.tensor_mul(x3s_, t3_, rws[k])
                    x2_ = actp.tile([128, TBLK], BF16, name=f"x2_{k}{f}",
                                    tag=f"x2_{k}{f}")
                    nc.vector.tensor_mul(x2_, sl_, x3s_)
                    x2s[k][f] = x2_
                    if k == K - 1:
                        xs_ = actp.tile([128, TBLK], BF16, name=f"xsum{f}",
                                        tag=f"xsum{f}")
                        nc.vector.tensor_add(xs_, x2s[0][f], x2s[1][f])
                        xsum[f] = xs_
                for er in range(NER):
                    ers = slice(er * 128, (er + 1) * 128)
                    psa2 = psp.tile([128, TBLK], F32, name="psa2", tag="pA")
                    for f in range(NF):
                        nc.tensor.matmul(psa2, a2s[f][:, ers], x2s[k][f],
                                         start=(f == 0), stop=(f == NF - 1))
                    m2_ = actp.tile([128, TBLK], BF16, name=f"m2_{k}{er}",
                                    tag=f"m2_{k}{er}")
                    nc.vector.tensor_mul(m2_, psa2, msk[k][er])
                    nc.sync.dma_start(out=m2o[k][er][:, tsl], in_=m2_)
                    m2[k][er] = m2_

            # ---- phase C: down-proj on the slot-summed activations ----
            for h in range(NH):
                hsl = slice(h * 128, (h + 1) * 128)
                psD = psp.tile([128, TBLK], F32, name="psD", tag="pD")
                for f in range(NF):
                    nc.tensor.matmul(psD, w2s[f][:, hsl], xsum[f],
                                     start=(f == 0), stop=(f == NF - 1))
                o_ = outp.tile([128, TBLK], F32, name="osb", tag="osb")
                nc.scalar.copy(o_, psD)
                nc.sync.dma_start(out=outT[h][:, tsl], in_=o_)

        if loop_cm is not None:
            loop_cm.__exit__(None, None, None)

    nc.finalize()
    return nc


def prepare_inputs(hidden_states, Wg, W1, W2, W3, A1, B1, A2, B2, A3, B3):
    """Host preprocessing: routing + per-core weight slicing/casting."""
    hidden_states, Wg, W1, W2, W3, A1, B1, A2, B2, A3, B3 = (
        np.asarray(a, dtype=np.float32)
        for a in (hidden_states, Wg, W1, W2, W3, A1, B1, A2, B2, A3, B3))
    x = np.ascontiguousarray(hidden_states.reshape(T, H))

    logits = x @ Wg.T.astype(np.float32)
    m = logits.max(-1, keepdims=True)
    p = np.exp(logits - m, dtype=np.float32)
    p /= p.sum(-1, keepdims=True)
    sel = np.argsort(-p, axis=-1, kind="stable")[:, :K]      # [T, K]
    rw = np.take_along_axis(p, sel, axis=1)
    rw = (rw / rw.sum(-1, keepdims=True)).astype(np.float32)  # [T, K]

    xT_np = np.ascontiguousarray(x.T).astype(NPBF16).reshape(NH, 128, T)

    # per-slot one-hot masks over the (e, r) axis, transposed to [ER, T]
    maskt_np = np.zeros((K, ER, T), dtype=NPBF16)
    for k in range(K):
        onehot = np.zeros((T, E), np.float32)
        onehot[np.arange(T), sel[:, k]] = 1.0
        maskt_np[k] = np.repeat(onehot, R, axis=1).T.astype(NPBF16)
    maskt_np = maskt_np.reshape(K, NER, 128, T)
    rwr_np = np.ascontiguousarray(rw.T).reshape(K, 1, T).astype(np.float32)

    # flattened LoRA tensors (full copies; small)
    A1f = A1.reshape(ER, H)                      # [er, H]
    A3f = A3.reshape(ER, H)
    B2f = B2.transpose(0, 2, 1).reshape(ER, H)   # [er, H]

    # per-slot masked LoRA down-projections, computed host-side in fp32
    a1_all = x @ A1f.T.astype(np.float32)        # [T, ER]
    a3_all = x @ A3f.T.astype(np.float32)
    m1t_np = np.zeros((K, ER, T), dtype=NPBF16)
    m3t_np = np.zeros((K, ER, T), dtype=NPBF16)
    for k in range(K):
        mx = np.repeat(
            np.eye(E, dtype=np.float32)[sel[:, k]], R, axis=1)   # [T, ER]
        m1t_np[k] = (a1_all * mx).T.astype(NPBF16)
        m3t_np[k] = (a3_all * mx).T.astype(NPBF16)
    m1t_np = m1t_np.reshape(K, NER, 128, T)
    m3t_np = m3t_np.reshape(K, NER, 128, T)

    in_maps = []
    for c in range(NCORES):
        fs = slice(c * FS, (c + 1) * FS)
        w1T = np.ascontiguousarray(W1[fs].T).astype(NPBF16)   # [H, FS]
        w3T = np.ascontiguousarray(W3[fs].T).astype(NPBF16)
        w1t_np = w1T.reshape(NH, 128, FS)
        w3t_np = w3T.reshape(NH, 128, FS)
        w2T = np.ascontiguousarray(W2[:, fs].T).astype(NPBF16)  # [FS, H]
        w2t_np = w2T.reshape(NF, 128, H)
        b1f = B1[:, fs, :].transpose(0, 2, 1).reshape(ER, FS)   # [er, f]
        b3f = B3[:, fs, :].transpose(0, 2, 1).reshape(ER, FS)
        b1t_np = np.ascontiguousarray(b1f).astype(NPBF16).reshape(NER, 128, FS)
        b3t_np = np.ascontiguousarray(b3f).astype(NPBF16).reshape(NER, 128, FS)
        a2f = A2[:, :, fs].reshape(ER, FS)                      # [er, f]
        a2t_np = np.ascontiguousarray(a2f.T).astype(NPBF16).reshape(NF, 128, ER)

        in_maps.append({
            "xT": xT_np, "w1t": w1t_np, "w3t": w3t_np, "w2t": w2t_np,
            "m1t": m1t_np, "m3t": m3t_np, "b1t": b1t_np, "b3t": b3t_np,
            "a2t": a2t_np, "maskt": maskt_np,
            "rwr": rwr_np,
        })
    return in_maps, B2f.astype(np.float32)


_CACHED_NC = None


def kernel(hidden_states, Wg, W1, W2, W3, A1, B1, A2, B2, A3, B3,
           _trace=False, _tmpdir=None):
    global _CACHED_NC
    in_maps, B2f = prepare_inputs(hidden_states, Wg, W1, W2, W3,
                                  A1, B1, A2, B2, A3, B3)
    if _CACHED_NC is None:
        _CACHED_NC = build_nc()
    nc = _CACHED_NC
    res = run_bass_kernel_spmd(nc, in_maps, list(range(NCORES)),
                               trace=_trace, tmpdir=_tmpdir)
    acc = np.zeros((NH, 128, T), np.float32)
    m2sum = np.zeros((K, ER, T), np.float32)
    for c in range(NCORES):
        acc += res.results[c]["outT"]
        m2sum += res.results[c]["m2o"].reshape(K, ER, T).astype(np.float32)
    out = acc.reshape(H, T)
    # host-side lora2: the final LoRA up-projection is linear, so it applies
    # to the core-summed masked activations in one small GEMM
    for k in range(K):
        out += B2f.T @ m2sum[k]
    out = out.T.reshape(B, S, H)
    kernel.last_results = res
    return out


if __name__ == "__main__":
    nc = build_nc()
    print("built ok")



# revision 15
# speedup vs baseline: 7.0217x; 7.0217x over previous
"""Mixtral sparse-MoE block with per-expert LoRA adapters on 8 Trainium2 cores.

Problem shapes: B=2, S=1024, H=2048, F=7168, E=8, R=32, top-K=2.
T = B*S = 2048 tokens.

Sharding: tensor-parallel over the FFN dim F. Core c owns rows
[c*896:(c+1)*896] of W1/W3 (and the matching B1/B3 LoRA rows) and the same
columns of W2/A2. Everything after the silu is linear in
x2s = silu(x1)*x3*rw, so each core emits an exact partial [H, T] output over
its F-shard and the host sums the 8 partials.

Work split (device vs host):
- Host: gating (softmax + top-2; 34M MACs), the tiny per-expert LoRA
  down-projections a1/a3 = x @ A{1,3}T masked per slot (m1/m3 uploads), and
  the final LoRA up-projection lora2 = B2 @ sum_cores(m2) (one small GEMM) -
  all the terminal-linear or pre-device pieces.
- Device (per core): base1/base3 = x @ W{1,3}[shard].T, the per-slot LoRA
  up-projections lora1/3 = B{1,3}[shard] @ m{1,3} accumulated onto the base
  in PSUM, silu/mul/scale chain, a2 = A2[shard] @ x2s with per-slot masking
  (m2, returned to host), and the down-projection on the slot-summed
  activations: down = W2[shard] @ (x2s_0 + x2s_1) - the two top-k slots share
  one down-proj because down is linear in x2s.

Layout is feature-major ([feature, token]) end to end so every matmul has its
contraction on the partition axis with zero on-device transposes. All matmul
operands are bf16 (fp32 matmul is 4x slower on TRN2); PSUM accumulates fp32.

Perf notes (measured on trn2 via an on-device For_i repeat loop):
- Each InstMatmult costs ~295ns at N=512 on this toolchain (LDWEIGHTS is
  emitted per-matmul by tile_legalize and does not overlap the matmul), so
  minimizing matmul count dominates: 1680 MMs -> ~475us/core.
- Every DMA instruction costs ~625ns on the shared HWDGE descriptor device,
  so weights are SBUF-resident and streamed tiles are [128, >=512].
"""

import sys
from contextlib import ExitStack

import numpy as np

try:
    import concourse.bass as bass  # noqa: F401
except ImportError:
    sys.path.insert(0, "/opt/trn_rl_repo")

import ml_dtypes

import concourse.bass as bass
import concourse.mybir as mybir
import concourse.tile as tile
from concourse import bacc
from concourse.bass_utils import run_bass_kernel_spmd

BF16 = mybir.dt.bfloat16
F32 = mybir.dt.float32
NPBF16 = ml_dtypes.bfloat16

B, S, H, F, E, R, K = 2, 1024, 2048, 7168, 8, 32, 2
T = B * S                      # 2048 tokens
ER = E * R                     # 256
NCORES = 8
FS = F // NCORES               # 896 per-core F shard
NH = H // 128                  # 16 h-chunks
NF = FS // 128                 # 7 f-chunks (per core)
NER = ER // 128                # 2 er-chunks
TBLK = 512
NT = T // TBLK                 # 4 token blocks


def build_nc(repeat=None):
    """Build the per-core Bass module.

    repeat=None emits the plain single-shot kernel (what the grader runs).
    repeat=n wraps the ENTIRE body — resident weight loads included — in an
    on-device For_i hardware loop for slope-based timing in test.py.
    """
    nc = bacc.Bacc(None)

    xT = nc.declare_dram_parameter("xT", [NH, 128, T], BF16, isOutput=False)
    w1t = nc.declare_dram_parameter("w1t", [NH, 128, FS], BF16, isOutput=False)
    w3t = nc.declare_dram_parameter("w3t", [NH, 128, FS], BF16, isOutput=False)
    w2t = nc.declare_dram_parameter("w2t", [NF, 128, H], BF16, isOutput=False)
    m1t = nc.declare_dram_parameter("m1t", [K, NER, 128, T], BF16, isOutput=False)
    m3t = nc.declare_dram_parameter("m3t", [K, NER, 128, T], BF16, isOutput=False)
    b1t = nc.declare_dram_parameter("b1t", [NER, 128, FS], BF16, isOutput=False)
    b3t = nc.declare_dram_parameter("b3t", [NER, 128, FS], BF16, isOutput=False)
    a2t = nc.declare_dram_parameter("a2t", [NF, 128, ER], BF16, isOutput=False)
    rwr = nc.declare_dram_parameter("rwr", [K, 1, T], BF16, isOutput=False)
    outT = nc.declare_dram_parameter("outT", [NH, 128, T], F32, isOutput=True)
    m2o = nc.declare_dram_parameter("m2o", [K, NER, 128, T], BF16, isOutput=True)

    with tile.TileContext(nc) as tc, ExitStack() as ctx:
        resw = ctx.enter_context(tc.tile_pool(name="resw", bufs=1))
        xsp = ctx.enter_context(tc.tile_pool(name="xsp", bufs=2))
        actp = ctx.enter_context(tc.tile_pool(name="actp", bufs=1))
        mp_ = ctx.enter_context(tc.tile_pool(name="mp", bufs=2))
        trans = ctx.enter_context(tc.tile_pool(name="trans", bufs=3))
        outp = ctx.enter_context(tc.tile_pool(name="outp", bufs=4))
        psp = ctx.enter_context(tc.tile_pool(name="psp", bufs=2, space="PSUM"))

        loop_cm = tc.For_i(0, repeat, 1) if repeat is not None else None
        if loop_cm is not None:
            loop_cm.__enter__()

        # ---- per-block input streamers ----
        def load_block_inputs(tb, xs=None):
            tsl = slice(tb * TBLK, (tb + 1) * TBLK)
            if xs is None:
                xs = []
                for h in range(NH):
                    xt_ = xsp.tile([128, TBLK], BF16, name=f"x{h}", tag=f"x{h}")
                    nc.sync.dma_start(out=xt_, in_=xT[h][:, tsl])
                    xs.append(xt_)
            m1 = [[None] * NER for _ in range(K)]
            m3 = [[None] * NER for _ in range(K)]
            for k in range(K):
                for er in range(NER):
                    m1_ = mp_.tile([128, TBLK], BF16, name=f"m1_{k}{er}",
                                   tag=f"m1_{k}{er}")
                    nc.scalar.dma_start(out=m1_, in_=m1t[k][er][:, tsl])
                    m1[k][er] = m1_
                    m3_ = mp_.tile([128, TBLK], BF16, name=f"m3_{k}{er}",
                                   tag=f"m3_{k}{er}")
                    nc.scalar.dma_start(out=m3_, in_=m3t[k][er][:, tsl])
                    m3[k][er] = m3_
            rws = []
            for k in range(K):
                r_ = mp_.tile([128, TBLK], BF16, name=f"rw{k}", tag=f"rw{k}")
                nc.scalar.dma_start(out=r_, in_=rwr[k][:, tsl].to_broadcast([128, TBLK]))
                rws.append(r_)
            return xs, rws, m1, m3

        # ---- resident weights, emitted in first-use order so the HWDGE
        # queue feeds phase A of block 0 as early as possible ----
        def resident(src, n, shape, nm, eng=None):
            ts = []
            for i in range(n):
                t_ = resw.tile(shape, BF16, name=f"{nm}{i}", tag=f"{nm}{i}")
                (eng or nc.sync).dma_start(out=t_, in_=src[i])
                ts.append(t_)
            return ts

        xs0 = []
        w1s, w3s = [], []
        for h in range(NH):
            xt_ = xsp.tile([128, TBLK], BF16, name=f"x{h}", tag=f"x{h}")
            nc.sync.dma_start(out=xt_, in_=xT[h][:, 0:TBLK])
            xs0.append(xt_)
            t1 = resw.tile([128, FS], BF16, name=f"w1s{h}", tag=f"w1s{h}")
            nc.sync.dma_start(out=t1, in_=w1t[h])
            w1s.append(t1)
            t3 = resw.tile([128, FS], BF16, name=f"w3s{h}", tag=f"w3s{h}")
            nc.sync.dma_start(out=t3, in_=w3t[h])
            w3s.append(t3)
        pre0 = load_block_inputs(0, xs0)
        b1s = resident(b1t, NER, [128, FS], "b1s", eng=nc.scalar)
        b3s = resident(b3t, NER, [128, FS], "b3s", eng=nc.scalar)
        w2s = resident(w2t, NF, [128, H], "w2s", eng=nc.scalar)
        a2s = resident(a2t, NF, [128, ER], "a2s", eng=nc.scalar)

        for tb in range(NT):
            tsl = slice(tb * TBLK, (tb + 1) * TBLK)
            xs, rws, m1, m3 = pre0 if tb == 0 else load_block_inputs(tb)

            # ---- phase A: shared base1/base3 and LoRA down-projections ----
            base1 = [None] * NF
            base3 = [None] * NF
            for f in range(NF):
                fsl = slice(f * 128, (f + 1) * 128)
                ps1 = psp.tile([128, TBLK], F32, name="ps1", tag="pA")
                ps3 = psp.tile([128, TBLK], F32, name="ps3", tag="pB")
                for h in range(NH):
                    nc.tensor.matmul(ps1, w1s[h][:, fsl], xs[h], start=(h == 0), stop=(h == NH - 1))
                    nc.tensor.matmul(ps3, w3s[h][:, fsl], xs[h], start=(h == 0), stop=(h == NH - 1))
                b1_ = actp.tile([128, TBLK], BF16, name=f"b1_{f}", tag=f"b1_{f}")
                nc.scalar.copy(b1_, ps1)
                base1[f] = b1_
                b3_ = actp.tile([128, TBLK], BF16, name=f"b3_{f}", tag=f"b3_{f}")
                nc.scalar.copy(b3_, ps3)
                base3[f] = b3_

            # ---- phase B: per-slot LoRA up-proj, silu, x2s; then a2 ----
            x2s = [[None] * NF for _ in range(K)]
            xsum = [None] * NF
            m2 = [[None] * NER for _ in range(K)]
            for k in range(K):
                for f in range(NF):
                    fsl = slice(f * 128, (f + 1) * 128)
                    psA = psp.tile([128, TBLK], F32, name="psA", tag="pA")
                    nc.tensor.matmul(psA, b1s[0][:, fsl], m1[k][0], start=True, stop=False)
                    nc.tensor.matmul(psA, b1s[1][:, fsl], m1[k][1], start=False, stop=True)
                    psB = psp.tile([128, TBLK], F32, name="psB", tag="pB")
                    nc.tensor.matmul(psB, b3s[0][:, fsl], m3[k][0], start=True, stop=False)
                    nc.tensor.matmul(psB, b3s[1][:, fsl], m3[k][1], start=False, stop=True)
                    t1_ = trans.tile([128, TBLK], BF16, name="t1", tag="t1")
                    nc.vector.tensor_add(t1_, psA, base1[f])
                    sl_ = trans.tile([128, TBLK], BF16, name="sl", tag="sl")
                    nc.scalar.activation(sl_, t1_, mybir.ActivationFunctionType.Silu)
                    t3_ = trans.tile([128, TBLK], BF16, name="t3", tag="t3")
                    nc.vector.tensor_add(t3_, psB, base3[f])
                    x3s_ = trans.tile([128, TBLK], BF16, name="x3s", tag="x3s")
                    nc.vector.tensor_mul(x3s_, t3_, rws[k])
                    x2_ = actp.tile([128, TBLK], BF16, name=f"x2_{k}{f}",
                                    tag=f"x2_{k}{f}")
                    nc.vector.tensor_mul(x2_, sl_, x3s_)
                    x2s[k][f] = x2_
                    if k == K - 1:
                        xs_ = actp.tile([128, TBLK], BF16, name=f"xsum{f}",
                                        tag=f"xsum{f}")
                        nc.vector.tensor_add(xs_, x2s[0][f], x2s[1][f])
                        xsum[f] = xs_
                for er in range(NER):
                    ers = slice(er * 128, (er + 1) * 128)
                    psa2 = psp.tile([128, TBLK], F32, name="psa2", tag="pA")
                    for f in range(NF):
                        nc.tensor.matmul(psa2, a2s[f][:, ers], x2s[k][f],
                                         start=(f == 0), stop=(f == NF - 1))
                    m2_ = actp.tile([128, TBLK], BF16, name=f"m2_{k}{er}",
                                    tag=f"m2_{k}{er}")
                    nc.vector.tensor_copy(m2_, psa2)
                    nc.gpsimd.dma_start(out=m2o[k][er][:, tsl], in_=m2_)
                    m2[k][er] = m2_

            # ---- phase C: down-proj on the slot-summed activations ----
            for h in range(NH):
                hsl = slice(h * 128, (h + 1) * 128)
                psD = psp.tile([128, TBLK], F32, name="psD", tag="pD")
                for f in range(NF):
                    nc.tensor.matmul(psD, w2s[f][:, hsl], xsum[f],
                                     start=(f == 0), stop=(f == NF - 1))
                o_ = outp.tile([128, TBLK], F32, name="osb", tag="osb")
                nc.scalar.copy(o_, psD)
                nc.gpsimd.dma_start(out=outT[h][:, tsl], in_=o_)

        if loop_cm is not None:
            loop_cm.__exit__(None, None, None)

    nc.finalize()
    return nc


def prepare_inputs(hidden_states, Wg, W1, W2, W3, A1, B1, A2, B2, A3, B3):
    """Host preprocessing: routing + per-core weight slicing/casting."""
    hidden_states, Wg, W1, W2, W3, A1, B1, A2, B2, A3, B3 = (
        np.asarray(a, dtype=np.float32)
        for a in (hidden_states, Wg, W1, W2, W3, A1, B1, A2, B2, A3, B3))
    x = np.ascontiguousarray(hidden_states.reshape(T, H))

    logits = x @ Wg.T.astype(np.float32)
    m = logits.max(-1, keepdims=True)
    p = np.exp(logits - m, dtype=np.float32)
    p /= p.sum(-1, keepdims=True)
    sel = np.argsort(-p, axis=-1, kind="stable")[:, :K]      # [T, K]
    rw = np.take_along_axis(p, sel, axis=1)
    rw = (rw / rw.sum(-1, keepdims=True)).astype(np.float32)  # [T, K]

    xT_np = np.ascontiguousarray(x.T).astype(NPBF16).reshape(NH, 128, T)

    # per-slot one-hot masks over the (e, r) axis, transposed to [ER, T];
    # applied HOST-side to the returned a2 (masking is elementwise, so it
    # commutes with the cross-core partial sum)
    masks = np.zeros((K, ER, T), dtype=np.float32)
    for k in range(K):
        onehot = np.zeros((T, E), np.float32)
        onehot[np.arange(T), sel[:, k]] = 1.0
        masks[k] = np.repeat(onehot, R, axis=1).T
    rwr_np = np.ascontiguousarray(rw.T).reshape(K, 1, T).astype(NPBF16)

    # flattened LoRA tensors (full copies; small)
    A1f = A1.reshape(ER, H)                      # [er, H]
    A3f = A3.reshape(ER, H)
    B2f = B2.transpose(0, 2, 1).reshape(ER, H)   # [er, H]

    # per-slot masked LoRA down-projections, computed host-side in fp32
    a1_all = x @ A1f.T.astype(np.float32)        # [T, ER]
    a3_all = x @ A3f.T.astype(np.float32)
    m1t_np = np.zeros((K, ER, T), dtype=NPBF16)
    m3t_np = np.zeros((K, ER, T), dtype=NPBF16)
    for k in range(K):
        mx = np.repeat(
            np.eye(E, dtype=np.float32)[sel[:, k]], R, axis=1)   # [T, ER]
        m1t_np[k] = (a1_all * mx).T.astype(NPBF16)
        m3t_np[k] = (a3_all * mx).T.astype(NPBF16)
    m1t_np = m1t_np.reshape(K, NER, 128, T)
    m3t_np = m3t_np.reshape(K, NER, 128, T)

    in_maps = []
    for c in range(NCORES):
        fs = slice(c * FS, (c + 1) * FS)
        w1T = np.ascontiguousarray(W1[fs].T).astype(NPBF16)   # [H, FS]
        w3T = np.ascontiguousarray(W3[fs].T).astype(NPBF16)
        w1t_np = w1T.reshape(NH, 128, FS)
        w3t_np = w3T.reshape(NH, 128, FS)
        w2T = np.ascontiguousarray(W2[:, fs].T).astype(NPBF16)  # [FS, H]
        w2t_np = w2T.reshape(NF, 128, H)
        b1f = B1[:, fs, :].transpose(0, 2, 1).reshape(ER, FS)   # [er, f]
        b3f = B3[:, fs, :].transpose(0, 2, 1).reshape(ER, FS)
        b1t_np = np.ascontiguousarray(b1f).astype(NPBF16).reshape(NER, 128, FS)
        b3t_np = np.ascontiguousarray(b3f).astype(NPBF16).reshape(NER, 128, FS)
        a2f = A2[:, :, fs].reshape(ER, FS)                      # [er, f]
        a2t_np = np.ascontiguousarray(a2f.T).astype(NPBF16).reshape(NF, 128, ER)

        in_maps.append({
            "xT": xT_np, "w1t": w1t_np, "w3t": w3t_np, "w2t": w2t_np,
            "m1t": m1t_np, "m3t": m3t_np, "b1t": b1t_np, "b3t": b3t_np,
            "a2t": a2t_np,
            "rwr": rwr_np,
        })
    return in_maps, (B2f.astype(np.float32), masks)


_CACHED_NC = None


def kernel(hidden_states, Wg, W1, W2, W3, A1, B1, A2, B2, A3, B3,
           _trace=False, _tmpdir=None):
    global _CACHED_NC
    in_maps, (B2f, masks) = prepare_inputs(hidden_states, Wg, W1, W2, W3,
                                           A1, B1, A2, B2, A3, B3)
    if _CACHED_NC is None:
        _CACHED_NC = build_nc()
    nc = _CACHED_NC
    res = run_bass_kernel_spmd(nc, in_maps, list(range(NCORES)),
                               trace=_trace, tmpdir=_tmpdir)
    acc = np.zeros((NH, 128, T), np.float32)
    m2sum = np.zeros((K, ER, T), np.float32)
    for c in range(NCORES):
        acc += res.results[c]["outT"]
        m2sum += res.results[c]["m2o"].reshape(K, ER, T).astype(np.float32)
    out = acc.reshape(H, T)
    # host-side lora2: mask the (unmasked, core-summed) a2, then the final
    # LoRA up-projection is linear -> one small GEMM per slot
    for k in range(K):
        out += B2f.T @ (m2sum[k] * masks[k])
    out = out.T.reshape(B, S, H)
    kernel.last_results = res
    return out


if __name__ == "__main__":
    nc = build_nc()
    print("built ok")

